# revision 22
# baseline (speedup 1.0000x reference)
"""Distributed Trainium2 kernel for CustomMultiHeadAttentionStoich (v3).

Sharding (8 cores): core c = (batch b=c//4, query slice p=c%4, 512 queries).
No collectives: each core computes the FULL K^T and V for its batch locally,
which removes the baseline's two serialized AllGathers (+barrier, ~160us).

All matmul operands are bf16: fp8 was measured to cost 5-8% output error
here (the softmax concentrates on few keys, so quantization noise does not
average out), far over the 2% budget.

Engine budget per core (at the 2.4 GHz sustained PE p-state):
 - PE ~260us: Q/K/V projections, scores (96-row contraction: 64 head dims +
   32 SVD feature rows for the stoichiometric bias), AV, out-projection.
 - ACT ~146us: exclusively exp over [128,2,512] two-PSUM-bank score pairs,
   with the 1/8 softmax scale folded into the activation scale operand.
 - DVE: staging copies, bias adds, reciprocal, normalization multiplies.

SBUF is tight: kat (per-head K^T+features, 64KB/partition) and qat are
resident; xq/xk/wo share one streaming pool, and xv/wv stream per V piece.
V pieces for head-groups 1-3 are spread one per attention slot, gated so a
group's writes are only emitted after the pool's reuse-distance readers.

Attention is software-pipelined flat over (head, chunk-pair) slots with the
AV matmuls lagging one slot so the in-order PE queue never waits on the exp
of the slot it just issued.
"""

import sys

sys.path.insert(0, "/opt/trn_rl_repo")

import numpy as np
import ml_dtypes

BF = ml_dtypes.bfloat16

B, T, D, H, DH = 2, 2048, 1024, 16, 64
NCORES = 8
TQ = 512  # queries per core
R = 16  # SVD rank per clamp-kernel half
AUG = 2 * R
CP = DH + AUG  # contraction rows for the scores matmul (96)
NGRID = 1024  # SVD grid
KC = T // 128  # 16 key chunks
VG = 65  # V columns per head incl. ones column
NCP = KC // 2  # 8 chunk pairs

_state = {}


def _features():
    """Rank-R SVD features of rc(x,y)=clip(x-y,0,0.2) on [0,1]^2."""
    if "grid" not in _state:
        g = (np.arange(NGRID) + 0.5) / NGRID
        M = np.clip(g[:, None] - g[None, :], 0.0, 0.2)
        U, S, Vt = np.linalg.svd(M, full_matrices=False)
        sc = np.sqrt(S[:R] * NGRID)
        _state["grid"] = g
        _state["phi"] = (U[:, :R] * sc).astype(np.float64)  # [NGRID, R] phi_j(x)
        _state["psi"] = (Vt[:R].T * sc).astype(np.float64)  # [NGRID, R] psi_j(y)
    return _state["grid"], _state["phi"], _state["psi"]


def _ev(tab, x):
    g = _state["grid"]
    return np.stack([np.interp(x, g, tab[:, j]) for j in range(R)])


def _build():
    if "nc" in _state:
        return _state["nc"]

    import concourse.bass as bass
    import concourse.mybir as mybir
    import concourse.tile as tile
    from concourse import bacc

    dt = mybir.dt
    ts = bass.ts
    ds = bass.ds

    nc = bacc.Bacc(
        "TRN2",
        target_bir_lowering=False,
        debug=False,
        num_devices=NCORES,
    )

    # ---- kernel I/O (per-core shards; host pre-slices/casts) ----
    xqE = nc.dram_tensor("xq", [128, 8, TQ], dt.bfloat16, kind="ExternalInput").ap()
    xkE = nc.dram_tensor("xk", [128, 8, T], dt.bfloat16, kind="ExternalInput").ap()
    xvE = nc.dram_tensor("xv", [128, 8, T], dt.bfloat16, kind="ExternalInput").ap()
    wqE = nc.dram_tensor("wq", [128, 8, D], dt.bfloat16, kind="ExternalInput").ap()
    wkE = nc.dram_tensor("wk", [128, 8, D], dt.bfloat16, kind="ExternalInput").ap()
    wvE = nc.dram_tensor(
        "wv", [128, 8, H * VG], dt.bfloat16, kind="ExternalInput"
    ).ap()
    woE = nc.dram_tensor("wo", [128, 8, D], dt.bfloat16, kind="ExternalInput").ap()
    bqE = nc.dram_tensor("bq", [128, 8, 1], dt.float32, kind="ExternalInput").ap()
    bkE = nc.dram_tensor("bk", [128, 8, 1], dt.float32, kind="ExternalInput").ap()
    bvE = nc.dram_tensor("bvA", [1, H * VG], dt.bfloat16, kind="ExternalInput").ap()
    boE = nc.dram_tensor("bo", [1, D], dt.bfloat16, kind="ExternalInput").ap()
    kfE = nc.dram_tensor("kfeat", [AUG, T], dt.bfloat16, kind="ExternalInput").ap()
    qfE = nc.dram_tensor(
        "qfeat", [H * AUG, TQ], dt.bfloat16, kind="ExternalInput"
    ).ap()
    outE = nc.dram_tensor("out", [TQ, D], dt.float32, kind="ExternalOutput").ap()

    Exp = mybir.ActivationFunctionType.Exp

    with tile.TileContext(nc) as tc:
        with (
            tc.tile_pool(name="consts", bufs=1) as consts,
            tc.tile_pool(name="dram", bufs=1, space="DRAM") as dram,
            tc.tile_pool(name="xs", bufs=2) as xsp,  # xq, xk/xv chunks, wo halves
            tc.tile_pool(name="vhp", bufs=3) as vhp,  # per-head V prefetch
            tc.tile_pool(name="ehp", bufs=3) as ehp,  # exp chunk-pair tiles
            tc.tile_pool(name="stage", bufs=3) as stp,
            tc.tile_pool(name="sty", bufs=4) as styp,
            tc.tile_pool(name="nrm", bufs=2) as nrm,
            tc.tile_pool(name="psA", bufs=2, space="PSUM") as psA,
            tc.tile_pool(name="psS", bufs=2, space="PSUM") as psS,
            tc.tile_pool(name="psAV", bufs=2, space="PSUM") as psAV,
        ):
            # ---- resident constants ----
            wq_sb = consts.tile([128, 8, D], dt.bfloat16, tag="wq", name="wq")
            wk_sb = consts.tile([128, 8, D], dt.bfloat16, tag="wk", name="wk")
            bq_sb = consts.tile([128, 8, 1], dt.float32, tag="bq", name="bq")
            bk_sb = consts.tile([128, 8, 1], dt.float32, tag="bk", name="bk")
            bv_sb = consts.tile([1, H * VG], dt.bfloat16, tag="bv", name="bv")
            bo_sb = consts.tile([1, D], dt.bfloat16, tag="bo", name="bo")
            ones_sb = consts.tile([1, 128], dt.bfloat16, tag="ones", name="ones")
            wv_sb = consts.tile([128, 8, H * VG], dt.bfloat16, tag="wv", name="wv")

            nc.sync.dma_start(out=wq_sb, in_=wqE)
            nc.sync.dma_start(out=wv_sb, in_=wvE)
            nc.sync.dma_start(out=bq_sb, in_=bqE)
            nc.sync.dma_start(out=wk_sb, in_=wkE)
            nc.sync.dma_start(out=bk_sb, in_=bkE)
            nc.sync.dma_start(out=bv_sb, in_=bvE)
            nc.sync.dma_start(out=bo_sb, in_=boE)
            nc.vector.memset(ones_sb, 1.0)

            qat = [
                consts.tile([CP, TQ], dt.bfloat16, tag=f"qat{h}", name=f"qat{h}")
                for h in range(H)
            ]
            kat = [
                consts.tile([CP, T], dt.bfloat16, tag=f"kat{h}", name=f"kat{h}")
                for h in range(H)
            ]
            aot2 = [
                consts.tile([128, TQ], dt.bfloat16, tag=f"aot{j}", name=f"aot{j}")
                for j in range(H // 2)
            ]

            vdram = dram.tile(
                [128, KC, H * VG], dt.bfloat16, tag="vdram", name="vdram"
            )
            for h in range(H):
                nc.sync.dma_start(out=kat[h][DH:CP, :], in_=kfE)
                nc.sync.dma_start(out=qat[h][DH:CP, :], in_=qfE[ds(AUG * h, AUG), :])

            # ---- Q projection ----
            xq_sb = xsp.tile([128, 8, TQ], dt.bfloat16, tag="xs", name="xq")
            nc.sync.dma_start(out=xq_sb, in_=xqE)
            for dc in range(8):
                ps = psA.tile([128, TQ], dt.float32, tag="mm", name="mmq")
                for kc in range(8):
                    nc.tensor.matmul(
                        ps,
                        lhsT=wq_sb[:, kc, ts(dc, 128)],
                        rhs=xq_sb[:, kc, :],
                        start=(kc == 0),
                        stop=(kc == 7),
                    )
                stg = stp.tile([128, TQ], dt.bfloat16, tag="stq", name="stq")
                nc.vector.tensor_scalar_add(stg, ps, bq_sb[:, dc, :])
                nc.sync.dma_start(out=qat[2 * dc][0:DH, :], in_=stg[0:DH, :])
                nc.sync.dma_start(out=qat[2 * dc + 1][0:DH, :], in_=stg[DH:128, :])

            # ---- K^T projection (full 2048 keys), xk streamed per 512 keys --
            for tc_i in range(4):
                xk_t = xsp.tile([128, 8, 512], dt.bfloat16, tag="xs", name="xkt")
                nc.sync.dma_start(out=xk_t, in_=xkE[:, :, ts(tc_i, 512)])
                for dc in range(8):
                    ps = psA.tile([128, 512], dt.float32, tag="mm", name="mmk")
                    for kc in range(8):
                        nc.tensor.matmul(
                            ps,
                            lhsT=wk_sb[:, kc, ts(dc, 128)],
                            rhs=xk_t[:, kc, :],
                            start=(kc == 0),
                            stop=(kc == 7),
                        )
                    nc.vector.tensor_scalar_add(
                        kat[2 * dc][0:DH, ts(tc_i, 512)],
                        ps[0:DH, :],
                        bk_sb[0:DH, dc, :],
                    )
                    stg = stp.tile([128, 512], dt.bfloat16, tag="stk", name="stk")
                    nc.vector.tensor_scalar_add(
                        stg[DH:128, :], ps[DH:128, :], bk_sb[DH:128, dc, :]
                    )
                    nc.sync.dma_start(
                        out=kat[2 * dc + 1][0:DH, ts(tc_i, 512)], in_=stg[DH:128, :]
                    )

            # ---- V projection (prelude, xv streamed once) -> vdram ----
            for t4 in range(4):
                xv_t = xsp.tile([128, 8, 512], dt.bfloat16, tag="xs", name="xvt")
                nc.sync.dma_start(out=xv_t, in_=xvE[:, :, ts(t4, 512)])
                for sub in range(4):
                    tc_i = 4 * t4 + sub
                    for g4 in range(4):
                        ps = psA.tile([128, 512], dt.float32, tag="mm", name="mmv")
                        for kc in range(8):
                            nc.tensor.matmul(
                                ps[:, 0 : 4 * VG],
                                lhsT=xv_t[:, kc, ts(sub, 128)],
                                rhs=wv_sb[:, kc, ts(g4, 4 * VG)],
                                start=(kc == 0),
                                stop=False,
                            )
                        nc.tensor.matmul(
                            ps[:, 0 : 4 * VG],
                            lhsT=ones_sb[:, :],
                            rhs=bv_sb[:, ts(g4, 4 * VG)],
                            start=False,
                            stop=True,
                        )
                        stv = stp.tile(
                            [128, 4 * VG], dt.bfloat16, tag="stv", name="stv"
                        )
                        nc.vector.tensor_copy(stv, ps[:, 0 : 4 * VG])
                        nc.sync.dma_start(
                            out=vdram[:, tc_i, ts(g4, 4 * VG)], in_=stv
                        )

            # per-head V prefetch out of vdram
            vh = [None] * H

            def emit_vh(h):
                vh[h] = vhp.tile([128, KC, VG], dt.bfloat16, tag="vh", name=f"vh{h}")
                nc.sync.dma_start(out=vh[h], in_=vdram[:, :, ds(VG * h, VG)])

            emit_vh(0)
            emit_vh(1)

            # ---- attention: flat pipeline over (head, chunk-pair) ----
            av_t = [None] * H
            eh_slot = {}
            pend_av = None  # (h, cp)
            pend_norm = None  # h, pre-norm due at next head's first slot
            pend_norm_pe = None  # h, PE broadcast due a few slots later
            norm_rcb = {}

            def emit_av(h, cp_i):
                eh2 = eh_slot.pop((h, cp_i))
                for j in range(2):
                    nc.tensor.matmul(
                        av_t[h],
                        lhsT=vh[h][:, 2 * cp_i + j, :],
                        rhs=eh2[:, j, :],
                        start=(cp_i == 0 and j == 0),
                        stop=(cp_i == NCP - 1 and j == 1),
                    )

            def emit_norm_pre(h):
                # den: PSUM row 64 -> SBUF (lane-aligned) -> DMA to partition 0,
                # then reciprocal at partition 0 (recip can't read PSUM@64 on HW)
                den65 = nrm.tile([VG, TQ], dt.float32, tag="den65", name="den65")
                nc.vector.tensor_copy(den65[DH:VG, :], av_t[h][DH:VG, :])
                den0 = nrm.tile([1, TQ], dt.float32, tag="den0", name="den0")
                nc.sync.dma_start(out=den0, in_=den65[DH:VG, :])
                rcf = nrm.tile([1, TQ], dt.float32, tag="rcf", name="rcf")
                nc.vector.reciprocal_approx_fast(rcf, den0)
                rcb0 = nrm.tile([1, TQ], dt.bfloat16, tag="rcb0", name="rcb0")
                nc.vector.tensor_copy(rcb0, rcf)
                norm_rcb[h] = rcb0

            def emit_norm_pe(h):
                # PE broadcast of rcb over 64 partitions, then DVE multiply
                ps_av = av_t[h]
                psb = psA.tile([DH, TQ], dt.float32, tag="mm", name="mmb")
                nc.tensor.matmul(
                    psb,
                    lhsT=ones_sb[:, 0:DH],
                    rhs=norm_rcb[h],
                    start=True,
                    stop=True,
                )
                bcb = nrm.tile([DH, TQ], dt.bfloat16, tag="bcb", name="bcb")
                nc.vector.tensor_copy(bcb, psb)
                if h % 2 == 0:
                    nc.vector.tensor_mul(
                        aot2[h // 2][0:DH, :], ps_av[0:DH, :], bcb
                    )
                else:
                    ot = nrm.tile([DH, TQ], dt.bfloat16, tag="ot", name="ot")
                    nc.vector.tensor_mul(ot, ps_av[0:DH, :], bcb)
                    nc.sync.dma_start(out=aot2[h // 2][DH:128, :], in_=ot)

            for h in range(H):
                av_t[h] = psAV.tile([VG, TQ], dt.float32, tag="av", name="av")
                for cp_i in range(NCP):
                    ps_s = psS.tile([128, 2, TQ], dt.float32, tag="s", name="s")
                    for j in range(2):
                        nc.tensor.matmul(
                            ps_s[:, j, :],
                            lhsT=kat[h][:, ts(2 * cp_i + j, 128)],
                            rhs=qat[h],
                            start=True,
                            stop=True,
                        )
                    eh2 = ehp.tile([128, 2, TQ], dt.bfloat16, tag="eh", name="eh")
                    nc.scalar.activation(eh2, ps_s, Exp, scale=0.125)
                    eh_slot[(h, cp_i)] = eh2
                    if pend_av is not None:
                        emit_av(*pend_av)
                        pend_av = None
                        if pend_norm is not None:
                            emit_norm_pre(pend_norm)
                            pend_norm_pe, pend_norm = pend_norm, None
                    if pend_norm_pe is not None and cp_i >= 3:
                        emit_norm_pe(pend_norm_pe)
                        pend_norm_pe = None
                    pend_av = (h, cp_i)
                    if cp_i == 2 and h + 2 < H:
                        emit_vh(h + 2)
                pend_norm = h
            emit_av(*pend_av)
            emit_norm_pre(pend_norm)
            emit_norm_pe(pend_norm)

            # ---- output projection (2 heads per 128-row chunk) ----
            wo_t = [
                xsp.tile([128, 8, 512], dt.bfloat16, tag="xs", name=f"wo{mc}")
                for mc in range(2)
            ]
            for mc in range(2):
                nc.sync.dma_start(out=wo_t[mc], in_=woE[:, :, ts(mc, 512)])
            for qc in range(4):
                for mc in range(2):
                    ps_y = psA.tile([128, 512], dt.float32, tag="mm", name="mmy")
                    for j in range(H // 2):
                        nc.tensor.matmul(
                            ps_y,
                            lhsT=aot2[j][:, ts(qc, 128)],
                            rhs=wo_t[mc][:, j, :],
                            start=(j == 0),
                            stop=False,
                        )
                    nc.tensor.matmul(
                        ps_y,
                        lhsT=ones_sb[:, :],
                        rhs=bo_sb[:, ts(mc, 512)],
                        start=False,
                        stop=True,
                    )
                    yst = styp.tile([128, 512], dt.float32, tag="sty", name="sty")
                    nc.vector.tensor_copy(yst, ps_y)
                    nc.sync.dma_start(out=outE[ts(qc, 128), ts(mc, 512)], in_=yst)

    nc.compile()
    _state["nc"] = nc
    return nc


def _pack8(mat):
    """[1024, N] -> [128, 8, N] with element [p, g, n] = mat[g*128+p, n]."""
    n = mat.shape[1]
    return np.ascontiguousarray(mat.reshape(8, 128, n).transpose(1, 0, 2))


def _make_in_maps(inputs):
    _features()
    gamma = float(np.float32(inputs["gamma"]))
    delta = float(np.float32(inputs["delta"]))
    ap_ = np.asarray(inputs["alpha_pos"], np.float64)
    an_ = np.asarray(inputs["alpha_neg"], np.float64)

    # exp() applies scale=1/8; gamma folds into Wq; qfeat compensates by 8.
    wqp = _pack8((np.asarray(inputs["Wq"], np.float64).T * gamma).astype(BF))
    bq = (np.asarray(inputs["bq"], np.float64) * gamma).astype(np.float32)
    bqp = np.ascontiguousarray(bq.reshape(8, 128).T)[:, :, None]
    wkp = _pack8(np.ascontiguousarray(np.asarray(inputs["Wk"]).T).astype(BF))
    bkp = np.ascontiguousarray(
        np.asarray(inputs["bk"], np.float32).reshape(8, 128).T
    )[:, :, None]
    wop = _pack8(np.ascontiguousarray(np.asarray(inputs["Wo"]).T).astype(BF))
    bo = np.asarray(inputs["bo"], np.float32)[None, :].astype(BF)

    wvT = np.asarray(inputs["Wv"], np.float64).T
    wvP = np.zeros((D, H * VG), np.float64)
    bvA = np.zeros((1, H * VG), np.float64)
    for h in range(H):
        wvP[:, VG * h : VG * h + DH] = wvT[:, DH * h : DH * h + DH]
        bvA[0, VG * h : VG * h + DH] = np.asarray(inputs["bv"], np.float64)[
            DH * h : DH * h + DH
        ]
        bvA[0, VG * h + DH] = 1.0
    wvp = _pack8(wvP.astype(BF))
    bvA = bvA.astype(BF)

    phi, psi = _state["phi"], _state["psi"]
    frac = np.asarray(inputs["frac"], np.float64)

    in_maps = []
    for c in range(NCORES):
        b, p = c // 4, c % 4
        fb = frac[b]
        fq = fb[TQ * p : TQ * (p + 1)]
        kfeat = np.concatenate([_ev(phi, fb), _ev(psi, fb)], 0).astype(BF)
        qfeat = np.zeros((H * AUG, TQ), np.float64)
        for h in range(H):
            a_h = 8.0 * delta * ap_[h] / NGRID
            b_h = -8.0 * delta * an_[h] / NGRID
            qfeat[AUG * h : AUG * h + R] = a_h * _ev(psi, fq)
            qfeat[AUG * h + R : AUG * (h + 1)] = b_h * _ev(phi, fq)
        qfeat = qfeat.astype(BF)

        xq = np.asarray(inputs["query"])[b, TQ * p : TQ * (p + 1)]
        in_maps.append(
            {
                "xq": _pack8(np.ascontiguousarray(xq.T).astype(BF)),
                "xk": _pack8(
                    np.ascontiguousarray(np.asarray(inputs["key"])[b].T).astype(BF)
                ),
                "xv": _pack8(
                    np.ascontiguousarray(np.asarray(inputs["value"])[b].T).astype(BF)
                ),
                "wq": wqp,
                "wk": wkp,
                "wv": wvp,
                "wo": wop,
                "bq": bqp,
                "bk": bkp,
                "bvA": bvA,
                "bo": bo,
                "kfeat": kfeat,
                "qfeat": qfeat,
            }
        )
    return in_maps


def _run(inputs, trace=False, **kw):
    from concourse.bass_utils import run_bass_kernel_spmd

    nc = _build()
    in_maps = _make_in_maps(inputs)
    res = run_bass_kernel_spmd(
        nc, in_maps, core_ids=list(range(NCORES)), trace=trace, **kw
    )
    out = np.zeros((B, T, D), np.float32)
    for c in range(NCORES):
        b, p = c // 4, c % 4
        out[b, TQ * p : TQ * (p + 1)] = res.results[c]["out"]
    return out, res


def kernel(**inputs):
    out, _ = _run(inputs)
    return out


# revision 24
# speedup vs baseline: 1.0951x; 1.0951x over previous
"""Distributed Trainium2 kernel for CustomMultiHeadAttentionStoich (v3).

Sharding (8 cores): core c = (batch b=c//4, query slice p=c%4, 512 queries).
No collectives: each core computes the FULL K^T and V for its batch locally,
which removes the baseline's two serialized AllGathers (+barrier, ~160us).

All matmul operands are bf16: fp8 was measured to cost 5-8% output error
here (the softmax concentrates on few keys, so quantization noise does not
average out), far over the 2% budget.

Engine budget per core (at the 2.4 GHz sustained PE p-state):
 - PE ~260us: Q/K/V projections, scores (96-row contraction: 64 head dims +
   32 SVD feature rows for the stoichiometric bias), AV, out-projection.
 - ACT ~146us: exclusively exp over [128,2,512] two-PSUM-bank score pairs,
   with the 1/8 softmax scale folded into the activation scale operand.
 - DVE: staging copies, bias adds, reciprocal, normalization multiplies.

SBUF is tight: kat (per-head K^T+features, 64KB/partition) and qat are
resident; xq/xk/wo share one streaming pool, and xv/wv stream per V piece.
V pieces for head-groups 1-3 are spread one per attention slot, gated so a
group's writes are only emitted after the pool's reuse-distance readers.

Attention is software-pipelined flat over (head, chunk-pair) slots with the
AV matmuls lagging one slot so the in-order PE queue never waits on the exp
of the slot it just issued.
"""

import sys

sys.path.insert(0, "/opt/trn_rl_repo")

import numpy as np
import ml_dtypes

BF = ml_dtypes.bfloat16

B, T, D, H, DH = 2, 2048, 1024, 16, 64
NCORES = 8
TQ = 512  # queries per core
R = 16  # SVD rank per clamp-kernel half
AUG = 2 * R
CP = DH + AUG  # contraction rows for the scores matmul (96)
NGRID = 1024  # SVD grid
KC = T // 128  # 16 key chunks
VG = 65  # V columns per head incl. ones column
NCP = KC // 2  # 8 chunk pairs

_state = {}


def _features():
    """Rank-R SVD features of rc(x,y)=clip(x-y,0,0.2) on [0,1]^2."""
    if "grid" not in _state:
        g = (np.arange(NGRID) + 0.5) / NGRID
        M = np.clip(g[:, None] - g[None, :], 0.0, 0.2)
        U, S, Vt = np.linalg.svd(M, full_matrices=False)
        sc = np.sqrt(S[:R] * NGRID)
        _state["grid"] = g
        _state["phi"] = (U[:, :R] * sc).astype(np.float64)  # [NGRID, R] phi_j(x)
        _state["psi"] = (Vt[:R].T * sc).astype(np.float64)  # [NGRID, R] psi_j(y)
    return _state["grid"], _state["phi"], _state["psi"]


def _ev(tab, x):
    g = _state["grid"]
    return np.stack([np.interp(x, g, tab[:, j]) for j in range(R)])


def _build():
    if "nc" in _state:
        return _state["nc"]

    import concourse.bass as bass
    import concourse.mybir as mybir
    import concourse.tile as tile
    from concourse import bacc

    dt = mybir.dt
    ts = bass.ts
    ds = bass.ds

    nc = bacc.Bacc(
        "TRN2",
        target_bir_lowering=False,
        debug=False,
        num_devices=NCORES,
    )

    # ---- kernel I/O (per-core shards; host pre-slices/casts) ----
    xqE = nc.dram_tensor("xq", [128, 8, TQ], dt.bfloat16, kind="ExternalInput").ap()
    xkE = nc.dram_tensor("xk", [128, 8, T], dt.bfloat16, kind="ExternalInput").ap()
    xvE = nc.dram_tensor("xv", [128, 8, T], dt.bfloat16, kind="ExternalInput").ap()
    wqE = nc.dram_tensor("wq", [128, 8, D], dt.bfloat16, kind="ExternalInput").ap()
    wkE = nc.dram_tensor("wk", [128, 8, D], dt.bfloat16, kind="ExternalInput").ap()
    wvE = nc.dram_tensor(
        "wv", [128, 8, H * VG], dt.bfloat16, kind="ExternalInput"
    ).ap()
    woE = nc.dram_tensor("wo", [128, 8, D], dt.bfloat16, kind="ExternalInput").ap()
    bqE = nc.dram_tensor("bq", [128, 8, 1], dt.float32, kind="ExternalInput").ap()
    bkE = nc.dram_tensor("bk", [128, 8, 1], dt.float32, kind="ExternalInput").ap()
    bvE = nc.dram_tensor("bvA", [1, H * VG], dt.bfloat16, kind="ExternalInput").ap()
    boE = nc.dram_tensor("bo", [1, D], dt.bfloat16, kind="ExternalInput").ap()
    kfE = nc.dram_tensor("kfeat", [AUG, T], dt.bfloat16, kind="ExternalInput").ap()
    qfE = nc.dram_tensor(
        "qfeat", [H * AUG, TQ], dt.bfloat16, kind="ExternalInput"
    ).ap()
    outE = nc.dram_tensor("out", [TQ, D], dt.float32, kind="ExternalOutput").ap()

    Exp = mybir.ActivationFunctionType.Exp

    with tile.TileContext(nc) as tc:
        with (
            tc.tile_pool(name="consts", bufs=1) as consts,
            tc.tile_pool(name="xs", bufs=2) as xsp,  # xq, xk/xv chunks, wo halves
            tc.tile_pool(name="wqs", bufs=2) as wqsp,  # wq streamed per dc
            tc.tile_pool(name="ehp", bufs=3) as ehp,  # exp chunk-pair tiles
            tc.tile_pool(name="stage", bufs=2) as stp,
            tc.tile_pool(name="sty", bufs=2) as styp,
            tc.tile_pool(name="nrm", bufs=2) as nrm,
            tc.tile_pool(name="nrm1", bufs=1) as nrm1,
            tc.tile_pool(name="psA", bufs=2, space="PSUM") as psA,
            tc.tile_pool(name="psS", bufs=2, space="PSUM") as psS,
            tc.tile_pool(name="psAV", bufs=2, space="PSUM") as psAV,
        ):
            # ---- resident constants ----
            wk_sb = consts.tile([128, 8, D], dt.bfloat16, tag="wk", name="wk")
            bq_sb = consts.tile([128, 8, 1], dt.float32, tag="bq", name="bq")
            bk_sb = consts.tile([128, 8, 1], dt.float32, tag="bk", name="bk")
            bv_sb = consts.tile([1, H * VG], dt.bfloat16, tag="bv", name="bv")
            bo_sb = consts.tile([1, D], dt.bfloat16, tag="bo", name="bo")
            ones_sb = consts.tile([1, 128], dt.bfloat16, tag="ones", name="ones")
            wv_sb = consts.tile([128, 8, H * VG], dt.bfloat16, tag="wv", name="wv")

            nc.sync.dma_start(out=wv_sb, in_=wvE)
            nc.sync.dma_start(out=bq_sb, in_=bqE)
            nc.sync.dma_start(out=wk_sb, in_=wkE)
            nc.sync.dma_start(out=bk_sb, in_=bkE)
            nc.sync.dma_start(out=bv_sb, in_=bvE)
            nc.sync.dma_start(out=bo_sb, in_=boE)
            nc.vector.memset(ones_sb, 1.0)

            qat = [
                consts.tile([CP, TQ], dt.bfloat16, tag=f"qat{h}", name=f"qat{h}")
                for h in range(H)
            ]
            kat = [
                consts.tile([CP, T], dt.bfloat16, tag=f"kat{h}", name=f"kat{h}")
                for h in range(H)
            ]
            aot2 = [
                consts.tile([128, TQ], dt.bfloat16, tag=f"aot{j}", name=f"aot{j}")
                for j in range(H // 2)
            ]

            vtl = consts.tile(
                [128, KC, H * VG], dt.bfloat16, tag="vtl", name="vtl"
            )
            for h in range(H):
                nc.sync.dma_start(out=kat[h][DH:CP, :], in_=kfE)
                nc.sync.dma_start(out=qat[h][DH:CP, :], in_=qfE[ds(AUG * h, AUG), :])

            # ---- Q projection ----
            xq_sb = xsp.tile([128, 8, TQ], dt.bfloat16, tag="xs", name="xq")
            nc.sync.dma_start(out=xq_sb, in_=xqE)
            for dc in range(8):
                wq_t = wqsp.tile([128, 8, 128], dt.bfloat16, tag="wq", name="wqt")
                nc.sync.dma_start(out=wq_t, in_=wqE[:, :, ts(dc, 128)])
                ps = psA.tile([128, TQ], dt.float32, tag="mm", name="mmq")
                for kc in range(8):
                    nc.tensor.matmul(
                        ps,
                        lhsT=wq_t[:, kc, :],
                        rhs=xq_sb[:, kc, :],
                        start=(kc == 0),
                        stop=(kc == 7),
                    )
                stg = stp.tile([128, TQ], dt.bfloat16, tag="stg", name="stq")
                nc.vector.tensor_scalar_add(stg, ps, bq_sb[:, dc, :])
                nc.sync.dma_start(out=qat[2 * dc][0:DH, :], in_=stg[0:DH, :])
                nc.sync.dma_start(out=qat[2 * dc + 1][0:DH, :], in_=stg[DH:128, :])

            # ---- K^T projection (full 2048 keys), xk streamed per 512 keys --
            for tc_i in range(4):
                xk_t = xsp.tile([128, 8, 512], dt.bfloat16, tag="xs", name="xkt")
                nc.sync.dma_start(out=xk_t, in_=xkE[:, :, ts(tc_i, 512)])
                for dc in range(8):
                    ps = psA.tile([128, 512], dt.float32, tag="mm", name="mmk")
                    for kc in range(8):
                        nc.tensor.matmul(
                            ps,
                            lhsT=wk_sb[:, kc, ts(dc, 128)],
                            rhs=xk_t[:, kc, :],
                            start=(kc == 0),
                            stop=(kc == 7),
                        )
                    nc.vector.tensor_scalar_add(
                        kat[2 * dc][0:DH, ts(tc_i, 512)],
                        ps[0:DH, :],
                        bk_sb[0:DH, dc, :],
                    )
                    stg = stp.tile([128, 512], dt.bfloat16, tag="stg", name="stk")
                    nc.vector.tensor_scalar_add(
                        stg[DH:128, :], ps[DH:128, :], bk_sb[DH:128, dc, :]
                    )
                    nc.sync.dma_start(
                        out=kat[2 * dc + 1][0:DH, ts(tc_i, 512)], in_=stg[DH:128, :]
                    )

            # ---- V projection (prelude, xv streamed once) -> vdram ----
            for t4 in range(4):
                xv_t = xsp.tile([128, 8, 512], dt.bfloat16, tag="xs", name="xvt")
                nc.sync.dma_start(out=xv_t, in_=xvE[:, :, ts(t4, 512)])
                for sub in range(4):
                    tc_i = 4 * t4 + sub
                    for g4 in range(4):
                        ps = psA.tile([128, 512], dt.float32, tag="mm", name="mmv")
                        for kc in range(8):
                            nc.tensor.matmul(
                                ps[:, 0 : 4 * VG],
                                lhsT=xv_t[:, kc, ts(sub, 128)],
                                rhs=wv_sb[:, kc, ts(g4, 4 * VG)],
                                start=(kc == 0),
                                stop=False,
                            )
                        nc.tensor.matmul(
                            ps[:, 0 : 4 * VG],
                            lhsT=ones_sb[:, :],
                            rhs=bv_sb[:, ts(g4, 4 * VG)],
                            start=False,
                            stop=True,
                        )
                        nc.vector.tensor_copy(
                            vtl[:, tc_i, ts(g4, 4 * VG)], ps[:, 0 : 4 * VG]
                        )

            # ---- attention: flat pipeline over (head, chunk-pair) ----
            av_t = [None] * H
            eh_slot = {}
            pend_av = None  # (h, cp)
            pend_norm = None  # h, pre-norm due at next head's first slot
            pend_norm_pe = None  # h, PE broadcast due a few slots later
            norm_rcb = {}

            def emit_av(h, cp_i):
                eh2 = eh_slot.pop((h, cp_i))
                for j in range(2):
                    nc.tensor.matmul(
                        av_t[h],
                        lhsT=vtl[:, 2 * cp_i + j, ds(VG * h, VG)],
                        rhs=eh2[:, j, :],
                        start=(cp_i == 0 and j == 0),
                        stop=(cp_i == NCP - 1 and j == 1),
                    )

            def emit_norm_pre(h):
                # den: PSUM row 64 -> SBUF (lane-aligned) -> DMA to partition 0,
                # then reciprocal at partition 0 (recip can't read PSUM@64 on HW)
                den65 = nrm1.tile([VG, TQ], dt.float32, tag="den65", name="den65")
                nc.vector.tensor_copy(den65[DH:VG, :], av_t[h][DH:VG, :])
                den0 = nrm1.tile([1, TQ], dt.float32, tag="den0", name="den0")
                nc.sync.dma_start(out=den0, in_=den65[DH:VG, :])
                rcf = nrm1.tile([1, TQ], dt.float32, tag="rcf", name="rcf")
                nc.vector.reciprocal_approx_fast(rcf, den0)
                rcb0 = nrm.tile([1, TQ], dt.bfloat16, tag="rcb0", name="rcb0")
                nc.vector.tensor_copy(rcb0, rcf)
                norm_rcb[h] = rcb0

            def emit_norm_pe(h):
                # PE broadcast of rcb over 64 partitions, then DVE multiply
                ps_av = av_t[h]
                psb = psA.tile([DH, TQ], dt.float32, tag="mm", name="mmb")
                nc.tensor.matmul(
                    psb,
                    lhsT=ones_sb[:, 0:DH],
                    rhs=norm_rcb[h],
                    start=True,
                    stop=True,
                )
                bcb = nrm.tile([DH, TQ], dt.bfloat16, tag="bcb", name="bcb")
                nc.vector.tensor_copy(bcb, psb)
                if h % 2 == 0:
                    nc.vector.tensor_mul(
                        aot2[h // 2][0:DH, :], ps_av[0:DH, :], bcb
                    )
                else:
                    ot = nrm.tile([DH, TQ], dt.bfloat16, tag="ot", name="ot")
                    nc.vector.tensor_mul(ot, ps_av[0:DH, :], bcb)
                    nc.sync.dma_start(out=aot2[h // 2][DH:128, :], in_=ot)

            for h in range(H):
                av_t[h] = psAV.tile([VG, TQ], dt.float32, tag="av", name="av")
                for cp_i in range(NCP):
                    ps_s = psS.tile([128, 2, TQ], dt.float32, tag="s", name="s")
                    for j in range(2):
                        nc.tensor.matmul(
                            ps_s[:, j, :],
                            lhsT=kat[h][:, ts(2 * cp_i + j, 128)],
                            rhs=qat[h],
                            start=True,
                            stop=True,
                        )
                    eh2 = ehp.tile([128, 2, TQ], dt.bfloat16, tag="eh", name="eh")
                    nc.scalar.activation(eh2, ps_s, Exp, scale=0.125)
                    eh_slot[(h, cp_i)] = eh2
                    if pend_av is not None:
                        emit_av(*pend_av)
                        pend_av = None
                        if pend_norm is not None:
                            emit_norm_pre(pend_norm)
                            pend_norm_pe, pend_norm = pend_norm, None
                    if pend_norm_pe is not None and cp_i >= 3:
                        emit_norm_pe(pend_norm_pe)
                        pend_norm_pe = None
                    pend_av = (h, cp_i)
                pend_norm = h
            emit_av(*pend_av)
            emit_norm_pre(pend_norm)
            emit_norm_pe(pend_norm)

            # ---- output projection (2 heads per 128-row chunk) ----
            wo_t = [
                xsp.tile([128, 8, 512], dt.bfloat16, tag="xs", name=f"wo{mc}")
                for mc in range(2)
            ]
            for mc in range(2):
                nc.sync.dma_start(out=wo_t[mc], in_=woE[:, :, ts(mc, 512)])
            for qc in range(4):
                for mc in range(2):
                    ps_y = psA.tile([128, 512], dt.float32, tag="mm", name="mmy")
                    for j in range(H // 2):
                        nc.tensor.matmul(
                            ps_y,
                            lhsT=aot2[j][:, ts(qc, 128)],
                            rhs=wo_t[mc][:, j, :],
                            start=(j == 0),
                            stop=False,
                        )
                    nc.tensor.matmul(
                        ps_y,
                        lhsT=ones_sb[:, :],
                        rhs=bo_sb[:, ts(mc, 512)],
                        start=False,
                        stop=True,
                    )
                    yst = styp.tile([128, 512], dt.float32, tag="sty", name="sty")
                    nc.vector.tensor_copy(yst, ps_y)
                    nc.sync.dma_start(out=outE[ts(qc, 128), ts(mc, 512)], in_=yst)

    nc.compile()
    _state["nc"] = nc
    return nc


def _pack8(mat):
    """[1024, N] -> [128, 8, N] with element [p, g, n] = mat[g*128+p, n]."""
    n = mat.shape[1]
    return np.ascontiguousarray(mat.reshape(8, 128, n).transpose(1, 0, 2))


def _make_in_maps(inputs):
    _features()
    gamma = float(np.float32(inputs["gamma"]))
    delta = float(np.float32(inputs["delta"]))
    ap_ = np.asarray(inputs["alpha_pos"], np.float64)
    an_ = np.asarray(inputs["alpha_neg"], np.float64)

    # exp() applies scale=1/8; gamma folds into Wq; qfeat compensates by 8.
    wqp = _pack8((np.asarray(inputs["Wq"], np.float64).T * gamma).astype(BF))
    bq = (np.asarray(inputs["bq"], np.float64) * gamma).astype(np.float32)
    bqp = np.ascontiguousarray(bq.reshape(8, 128).T)[:, :, None]
    wkp = _pack8(np.ascontiguousarray(np.asarray(inputs["Wk"]).T).astype(BF))
    bkp = np.ascontiguousarray(
        np.asarray(inputs["bk"], np.float32).reshape(8, 128).T
    )[:, :, None]
    wop = _pack8(np.ascontiguousarray(np.asarray(inputs["Wo"]).T).astype(BF))
    bo = np.asarray(inputs["bo"], np.float32)[None, :].astype(BF)

    wvT = np.asarray(inputs["Wv"], np.float64).T
    wvP = np.zeros((D, H * VG), np.float64)
    bvA = np.zeros((1, H * VG), np.float64)
    for h in range(H):
        wvP[:, VG * h : VG * h + DH] = wvT[:, DH * h : DH * h + DH]
        bvA[0, VG * h : VG * h + DH] = np.asarray(inputs["bv"], np.float64)[
            DH * h : DH * h + DH
        ]
        bvA[0, VG * h + DH] = 1.0
    wvp = _pack8(wvP.astype(BF))
    bvA = bvA.astype(BF)

    phi, psi = _state["phi"], _state["psi"]
    frac = np.asarray(inputs["frac"], np.float64)

    in_maps = []
    for c in range(NCORES):
        b, p = c // 4, c % 4
        fb = frac[b]
        fq = fb[TQ * p : TQ * (p + 1)]
        kfeat = np.concatenate([_ev(phi, fb), _ev(psi, fb)], 0).astype(BF)
        qfeat = np.zeros((H * AUG, TQ), np.float64)
        for h in range(H):
            a_h = 8.0 * delta * ap_[h] / NGRID
            b_h = -8.0 * delta * an_[h] / NGRID
            qfeat[AUG * h : AUG * h + R] = a_h * _ev(psi, fq)
            qfeat[AUG * h + R : AUG * (h + 1)] = b_h * _ev(phi, fq)
        qfeat = qfeat.astype(BF)

        xq = np.asarray(inputs["query"])[b, TQ * p : TQ * (p + 1)]
        in_maps.append(
            {
                "xq": _pack8(np.ascontiguousarray(xq.T).astype(BF)),
                "xk": _pack8(
                    np.ascontiguousarray(np.asarray(inputs["key"])[b].T).astype(BF)
                ),
                "xv": _pack8(
                    np.ascontiguousarray(np.asarray(inputs["value"])[b].T).astype(BF)
                ),
                "wq": wqp,
                "wk": wkp,
                "wv": wvp,
                "wo": wop,
                "bq": bqp,
                "bk": bkp,
                "bvA": bvA,
                "bo": bo,
                "kfeat": kfeat,
                "qfeat": qfeat,
            }
        )
    return in_maps


def _run(inputs, trace=False, **kw):
    from concourse.bass_utils import run_bass_kernel_spmd

    nc = _build()
    in_maps = _make_in_maps(inputs)
    res = run_bass_kernel_spmd(
        nc, in_maps, core_ids=list(range(NCORES)), trace=trace, **kw
    )
    out = np.zeros((B, T, D), np.float32)
    for c in range(NCORES):
        b, p = c // 4, c % 4
        out[b, TQ * p : TQ * (p + 1)] = res.results[c]["out"]
    return out, res


def kernel(**inputs):
    out, _ = _run(inputs)
    return out


# revision 25
# speedup vs baseline: 1.1225x; 1.0250x over previous
"""Distributed Trainium2 kernel for CustomMultiHeadAttentionStoich (v3).

Sharding (8 cores): core c = (batch b=c//4, query slice p=c%4, 512 queries).
No collectives: each core computes the FULL K^T and V for its batch locally,
which removes the baseline's two serialized AllGathers (+barrier, ~160us).

All matmul operands are bf16: fp8 was measured to cost 5-8% output error
here (the softmax concentrates on few keys, so quantization noise does not
average out), far over the 2% budget.

Engine budget per core (at the 2.4 GHz sustained PE p-state):
 - PE ~260us: Q/K/V projections, scores (96-row contraction: 64 head dims +
   32 SVD feature rows for the stoichiometric bias), AV, out-projection.
 - ACT ~146us: exclusively exp over [128,2,512] two-PSUM-bank score pairs,
   with the 1/8 softmax scale folded into the activation scale operand.
 - DVE: staging copies, bias adds, reciprocal, normalization multiplies.

SBUF is tight: kat (per-head K^T+features, 64KB/partition) and qat are
resident; xq/xk/wo share one streaming pool, and xv/wv stream per V piece.
V pieces for head-groups 1-3 are spread one per attention slot, gated so a
group's writes are only emitted after the pool's reuse-distance readers.

Attention is software-pipelined flat over (head, chunk-pair) slots with the
AV matmuls lagging one slot so the in-order PE queue never waits on the exp
of the slot it just issued.
"""

import sys

sys.path.insert(0, "/opt/trn_rl_repo")

import numpy as np
import ml_dtypes

BF = ml_dtypes.bfloat16

B, T, D, H, DH = 2, 2048, 1024, 16, 64
NCORES = 8
TQ = 512  # queries per core
R = 16  # SVD rank per clamp-kernel half
AUG = 2 * R
CP = DH + AUG  # contraction rows for the scores matmul (96)
NGRID = 1024  # SVD grid
KC = T // 128  # 16 key chunks
VG = 65  # V columns per head incl. ones column
NCP = KC // 2  # 8 chunk pairs

_state = {}


def _features():
    """Rank-R SVD features of rc(x,y)=clip(x-y,0,0.2) on [0,1]^2."""
    if "grid" not in _state:
        g = (np.arange(NGRID) + 0.5) / NGRID
        M = np.clip(g[:, None] - g[None, :], 0.0, 0.2)
        U, S, Vt = np.linalg.svd(M, full_matrices=False)
        sc = np.sqrt(S[:R] * NGRID)
        _state["grid"] = g
        _state["phi"] = (U[:, :R] * sc).astype(np.float64)  # [NGRID, R] phi_j(x)
        _state["psi"] = (Vt[:R].T * sc).astype(np.float64)  # [NGRID, R] psi_j(y)
    return _state["grid"], _state["phi"], _state["psi"]


def _ev(tab, x):
    g = _state["grid"]
    return np.stack([np.interp(x, g, tab[:, j]) for j in range(R)])


def _build():
    if "nc" in _state:
        return _state["nc"]

    import concourse.bass as bass
    import concourse.mybir as mybir
    import concourse.tile as tile
    from concourse import bacc

    dt = mybir.dt
    ts = bass.ts
    ds = bass.ds

    nc = bacc.Bacc(
        "TRN2",
        target_bir_lowering=False,
        debug=False,
        num_devices=NCORES,
    )

    # ---- kernel I/O (per-core shards; host pre-slices/casts) ----
    xqE = nc.dram_tensor("xq", [128, 8, TQ], dt.bfloat16, kind="ExternalInput").ap()
    xkE = nc.dram_tensor("xk", [128, 8, T], dt.bfloat16, kind="ExternalInput").ap()
    xvE = nc.dram_tensor("xv", [128, 8, T], dt.bfloat16, kind="ExternalInput").ap()
    wqE = nc.dram_tensor("wq", [128, 8, D], dt.bfloat16, kind="ExternalInput").ap()
    wkE = nc.dram_tensor("wk", [128, 8, D], dt.bfloat16, kind="ExternalInput").ap()
    wvE = nc.dram_tensor(
        "wv", [128, 8, H * VG], dt.bfloat16, kind="ExternalInput"
    ).ap()
    woE = nc.dram_tensor("wo", [128, 8, D], dt.bfloat16, kind="ExternalInput").ap()
    bqE = nc.dram_tensor("bq", [128, 8, 1], dt.float32, kind="ExternalInput").ap()
    bkE = nc.dram_tensor("bk", [128, 8, 1], dt.float32, kind="ExternalInput").ap()
    bvE = nc.dram_tensor("bvA", [1, H * VG], dt.bfloat16, kind="ExternalInput").ap()
    boE = nc.dram_tensor("bo", [1, D], dt.bfloat16, kind="ExternalInput").ap()
    kfE = nc.dram_tensor("kfeat", [AUG, T], dt.bfloat16, kind="ExternalInput").ap()
    qfE = nc.dram_tensor(
        "qfeat", [H * AUG, TQ], dt.bfloat16, kind="ExternalInput"
    ).ap()
    outE = nc.dram_tensor("out", [TQ, D], dt.float32, kind="ExternalOutput").ap()

    Exp = mybir.ActivationFunctionType.Exp

    with tile.TileContext(nc) as tc:
        with (
            tc.tile_pool(name="consts", bufs=1) as consts,
            tc.tile_pool(name="xs", bufs=2) as xsp,  # xq, xk/xv chunks, wo halves
            tc.tile_pool(name="wqs", bufs=4) as wqsp,  # wq streamed per dc
            tc.tile_pool(name="ehp", bufs=3) as ehp,  # exp chunk-pair tiles
            tc.tile_pool(name="stage", bufs=2) as stp,
            tc.tile_pool(name="sty", bufs=2) as styp,
            tc.tile_pool(name="nrm", bufs=2) as nrm,
            tc.tile_pool(name="nrm1", bufs=1) as nrm1,
            tc.tile_pool(name="psA", bufs=2, space="PSUM") as psA,
            tc.tile_pool(name="psS", bufs=2, space="PSUM") as psS,
            tc.tile_pool(name="psAV", bufs=2, space="PSUM") as psAV,
        ):
            # ---- resident constants ----
            wk_sb = consts.tile([128, 8, D], dt.bfloat16, tag="wk", name="wk")
            bq_sb = consts.tile([128, 8, 1], dt.float32, tag="bq", name="bq")
            bk_sb = consts.tile([128, 8, 1], dt.float32, tag="bk", name="bk")
            bv_sb = consts.tile([1, H * VG], dt.bfloat16, tag="bv", name="bv")
            bo_sb = consts.tile([1, D], dt.bfloat16, tag="bo", name="bo")
            ones_sb = consts.tile([1, 128], dt.bfloat16, tag="ones", name="ones")
            wv_sb = consts.tile([128, 8, H * VG], dt.bfloat16, tag="wv", name="wv")

            nc.gpsimd.dma_start(out=wv_sb, in_=wvE)
            nc.gpsimd.dma_start(out=bq_sb, in_=bqE)
            nc.gpsimd.dma_start(out=wk_sb, in_=wkE)
            nc.gpsimd.dma_start(out=bk_sb, in_=bkE)
            nc.gpsimd.dma_start(out=bv_sb, in_=bvE)
            nc.gpsimd.dma_start(out=bo_sb, in_=boE)
            nc.vector.memset(ones_sb, 1.0)

            qat = [
                consts.tile([CP, TQ], dt.bfloat16, tag=f"qat{h}", name=f"qat{h}")
                for h in range(H)
            ]
            kat = [
                consts.tile([CP, T], dt.bfloat16, tag=f"kat{h}", name=f"kat{h}")
                for h in range(H)
            ]
            aot2 = [
                consts.tile([128, TQ], dt.bfloat16, tag=f"aot{j}", name=f"aot{j}")
                for j in range(H // 2)
            ]

            vtl = consts.tile(
                [128, KC, H * VG], dt.bfloat16, tag="vtl", name="vtl"
            )
            for h in range(H):
                nc.gpsimd.dma_start(out=kat[h][DH:CP, :], in_=kfE)
                nc.gpsimd.dma_start(out=qat[h][DH:CP, :], in_=qfE[ds(AUG * h, AUG), :])

            # ---- Q projection ----
            xq_sb = xsp.tile([128, 8, TQ], dt.bfloat16, tag="xs", name="xq")
            nc.sync.dma_start(out=xq_sb, in_=xqE)
            wq_t = {}

            def load_wq(dc):
                wq_t[dc] = wqsp.tile(
                    [128, 8, 128], dt.bfloat16, tag="wq", name="wqt"
                )
                nc.sync.dma_start(out=wq_t[dc], in_=wqE[:, :, ts(dc, 128)])

            for dc in range(4):
                load_wq(dc)
            for dc in range(8):
                if dc + 4 < 8:
                    load_wq(dc + 4)
                ps = psA.tile([128, TQ], dt.float32, tag="mm", name="mmq")
                for kc in range(8):
                    nc.tensor.matmul(
                        ps,
                        lhsT=wq_t[dc][:, kc, :],
                        rhs=xq_sb[:, kc, :],
                        start=(kc == 0),
                        stop=(kc == 7),
                    )
                stg = stp.tile([128, TQ], dt.bfloat16, tag="stg", name="stq")
                nc.vector.tensor_scalar_add(stg, ps, bq_sb[:, dc, :])
                nc.sync.dma_start(out=qat[2 * dc][0:DH, :], in_=stg[0:DH, :])
                nc.sync.dma_start(out=qat[2 * dc + 1][0:DH, :], in_=stg[DH:128, :])

            # ---- K^T projection (full 2048 keys), xk streamed per 512 keys --
            for tc_i in range(4):
                xk_t = xsp.tile([128, 8, 512], dt.bfloat16, tag="xs", name="xkt")
                nc.scalar.dma_start(out=xk_t, in_=xkE[:, :, ts(tc_i, 512)])
                for dc in range(8):
                    ps = psA.tile([128, 512], dt.float32, tag="mm", name="mmk")
                    for kc in range(8):
                        nc.tensor.matmul(
                            ps,
                            lhsT=wk_sb[:, kc, ts(dc, 128)],
                            rhs=xk_t[:, kc, :],
                            start=(kc == 0),
                            stop=(kc == 7),
                        )
                    nc.vector.tensor_scalar_add(
                        kat[2 * dc][0:DH, ts(tc_i, 512)],
                        ps[0:DH, :],
                        bk_sb[0:DH, dc, :],
                    )
                    stg = stp.tile([128, 512], dt.bfloat16, tag="stg", name="stk")
                    nc.vector.tensor_scalar_add(
                        stg[DH:128, :], ps[DH:128, :], bk_sb[DH:128, dc, :]
                    )
                    nc.sync.dma_start(
                        out=kat[2 * dc + 1][0:DH, ts(tc_i, 512)], in_=stg[DH:128, :]
                    )

            # ---- V projection (prelude, xv streamed once) -> vdram ----
            for t4 in range(4):
                xv_t = xsp.tile([128, 8, 512], dt.bfloat16, tag="xs", name="xvt")
                nc.scalar.dma_start(out=xv_t, in_=xvE[:, :, ts(t4, 512)])
                for sub in range(4):
                    tc_i = 4 * t4 + sub
                    for g4 in range(4):
                        ps = psA.tile([128, 512], dt.float32, tag="mm", name="mmv")
                        for kc in range(8):
                            nc.tensor.matmul(
                                ps[:, 0 : 4 * VG],
                                lhsT=xv_t[:, kc, ts(sub, 128)],
                                rhs=wv_sb[:, kc, ts(g4, 4 * VG)],
                                start=(kc == 0),
                                stop=False,
                            )
                        nc.tensor.matmul(
                            ps[:, 0 : 4 * VG],
                            lhsT=ones_sb[:, :],
                            rhs=bv_sb[:, ts(g4, 4 * VG)],
                            start=False,
                            stop=True,
                        )
                        nc.vector.tensor_copy(
                            vtl[:, tc_i, ts(g4, 4 * VG)], ps[:, 0 : 4 * VG]
                        )

            # ---- attention: flat pipeline over (head, chunk-pair) ----
            av_t = [None] * H
            eh_slot = {}
            pend_av = None  # (h, cp)
            pend_norm = None  # h, pre-norm due at next head's first slot
            pend_norm_pe = None  # h, PE broadcast due a few slots later
            norm_rcb = {}

            def emit_av(h, cp_i):
                eh2 = eh_slot.pop((h, cp_i))
                for j in range(2):
                    nc.tensor.matmul(
                        av_t[h],
                        lhsT=vtl[:, 2 * cp_i + j, ds(VG * h, VG)],
                        rhs=eh2[:, j, :],
                        start=(cp_i == 0 and j == 0),
                        stop=(cp_i == NCP - 1 and j == 1),
                    )

            def emit_norm_pre(h):
                # den: PSUM row 64 -> SBUF (lane-aligned) -> DMA to partition 0,
                # then reciprocal at partition 0 (recip can't read PSUM@64 on HW)
                den65 = nrm1.tile([VG, TQ], dt.float32, tag="den65", name="den65")
                nc.vector.tensor_copy(den65[DH:VG, :], av_t[h][DH:VG, :])
                den0 = nrm1.tile([1, TQ], dt.float32, tag="den0", name="den0")
                nc.sync.dma_start(out=den0, in_=den65[DH:VG, :])
                rcf = nrm1.tile([1, TQ], dt.float32, tag="rcf", name="rcf")
                nc.vector.reciprocal_approx_fast(rcf, den0)
                rcb0 = nrm.tile([1, TQ], dt.bfloat16, tag="rcb0", name="rcb0")
                nc.vector.tensor_copy(rcb0, rcf)
                norm_rcb[h] = rcb0

            def emit_norm_pe(h):
                # PE broadcast of rcb over 64 partitions, then DVE multiply
                ps_av = av_t[h]
                psb = psA.tile([DH, TQ], dt.float32, tag="mm", name="mmb")
                nc.tensor.matmul(
                    psb,
                    lhsT=ones_sb[:, 0:DH],
                    rhs=norm_rcb[h],
                    start=True,
                    stop=True,
                )
                bcb = nrm.tile([DH, TQ], dt.bfloat16, tag="bcb", name="bcb")
                nc.vector.tensor_copy(bcb, psb)
                if h % 2 == 0:
                    nc.vector.tensor_mul(
                        aot2[h // 2][0:DH, :], ps_av[0:DH, :], bcb
                    )
                else:
                    ot = nrm.tile([DH, TQ], dt.bfloat16, tag="ot", name="ot")
                    nc.vector.tensor_mul(ot, ps_av[0:DH, :], bcb)
                    nc.sync.dma_start(out=aot2[h // 2][DH:128, :], in_=ot)

            for h in range(H):
                av_t[h] = psAV.tile([VG, TQ], dt.float32, tag="av", name="av")
                for cp_i in range(NCP):
                    ps_s = psS.tile([128, 2, TQ], dt.float32, tag="s", name="s")
                    for j in range(2):
                        nc.tensor.matmul(
                            ps_s[:, j, :],
                            lhsT=kat[h][:, ts(2 * cp_i + j, 128)],
                            rhs=qat[h],
                            start=True,
                            stop=True,
                        )
                    eh2 = ehp.tile([128, 2, TQ], dt.bfloat16, tag="eh", name="eh")
                    nc.scalar.activation(eh2, ps_s, Exp, scale=0.125)
                    eh_slot[(h, cp_i)] = eh2
                    if pend_av is not None:
                        emit_av(*pend_av)
                        pend_av = None
                        if pend_norm is not None:
                            emit_norm_pre(pend_norm)
                            pend_norm_pe, pend_norm = pend_norm, None
                    if pend_norm_pe is not None and cp_i >= 3:
                        emit_norm_pe(pend_norm_pe)
                        pend_norm_pe = None
                    pend_av = (h, cp_i)
                pend_norm = h
            emit_av(*pend_av)
            emit_norm_pre(pend_norm)
            emit_norm_pe(pend_norm)

            # ---- output projection (2 heads per 128-row chunk) ----
            wo_t = [
                xsp.tile([128, 8, 512], dt.bfloat16, tag="xs", name=f"wo{mc}")
                for mc in range(2)
            ]
            for mc in range(2):
                nc.sync.dma_start(out=wo_t[mc], in_=woE[:, :, ts(mc, 512)])
            for qc in range(4):
                for mc in range(2):
                    ps_y = psA.tile([128, 512], dt.float32, tag="mm", name="mmy")
                    for j in range(H // 2):
                        nc.tensor.matmul(
                            ps_y,
                            lhsT=aot2[j][:, ts(qc, 128)],
                            rhs=wo_t[mc][:, j, :],
                            start=(j == 0),
                            stop=False,
                        )
                    nc.tensor.matmul(
                        ps_y,
                        lhsT=ones_sb[:, :],
                        rhs=bo_sb[:, ts(mc, 512)],
                        start=False,
                        stop=True,
                    )
                    yst = styp.tile([128, 512], dt.float32, tag="sty", name="sty")
                    nc.vector.tensor_copy(yst, ps_y)
                    nc.sync.dma_start(out=outE[ts(qc, 128), ts(mc, 512)], in_=yst)

    nc.compile()
    _state["nc"] = nc
    return nc


def _pack8(mat):
    """[1024, N] -> [128, 8, N] with element [p, g, n] = mat[g*128+p, n]."""
    n = mat.shape[1]
    return np.ascontiguousarray(mat.reshape(8, 128, n).transpose(1, 0, 2))


def _make_in_maps(inputs):
    _features()
    gamma = float(np.float32(inputs["gamma"]))
    delta = float(np.float32(inputs["delta"]))
    ap_ = np.asarray(inputs["alpha_pos"], np.float64)
    an_ = np.asarray(inputs["alpha_neg"], np.float64)

    # exp() applies scale=1/8; gamma folds into Wq; qfeat compensates by 8.
    wqp = _pack8((np.asarray(inputs["Wq"], np.float64).T * gamma).astype(BF))
    bq = (np.asarray(inputs["bq"], np.float64) * gamma).astype(np.float32)
    bqp = np.ascontiguousarray(bq.reshape(8, 128).T)[:, :, None]
    wkp = _pack8(np.ascontiguousarray(np.asarray(inputs["Wk"]).T).astype(BF))
    bkp = np.ascontiguousarray(
        np.asarray(inputs["bk"], np.float32).reshape(8, 128).T
    )[:, :, None]
    wop = _pack8(np.ascontiguousarray(np.asarray(inputs["Wo"]).T).astype(BF))
    bo = np.asarray(inputs["bo"], np.float32)[None, :].astype(BF)

    wvT = np.asarray(inputs["Wv"], np.float64).T
    wvP = np.zeros((D, H * VG), np.float64)
    bvA = np.zeros((1, H * VG), np.float64)
    for h in range(H):
        wvP[:, VG * h : VG * h + DH] = wvT[:, DH * h : DH * h + DH]
        bvA[0, VG * h : VG * h + DH] = np.asarray(inputs["bv"], np.float64)[
            DH * h : DH * h + DH
        ]
        bvA[0, VG * h + DH] = 1.0
    wvp = _pack8(wvP.astype(BF))
    bvA = bvA.astype(BF)

    phi, psi = _state["phi"], _state["psi"]
    frac = np.asarray(inputs["frac"], np.float64)

    in_maps = []
    for c in range(NCORES):
        b, p = c // 4, c % 4
        fb = frac[b]
        fq = fb[TQ * p : TQ * (p + 1)]
        kfeat = np.concatenate([_ev(phi, fb), _ev(psi, fb)], 0).astype(BF)
        qfeat = np.zeros((H * AUG, TQ), np.float64)
        for h in range(H):
            a_h = 8.0 * delta * ap_[h] / NGRID
            b_h = -8.0 * delta * an_[h] / NGRID
            qfeat[AUG * h : AUG * h + R] = a_h * _ev(psi, fq)
            qfeat[AUG * h + R : AUG * (h + 1)] = b_h * _ev(phi, fq)
        qfeat = qfeat.astype(BF)

        xq = np.asarray(inputs["query"])[b, TQ * p : TQ * (p + 1)]
        in_maps.append(
            {
                "xq": _pack8(np.ascontiguousarray(xq.T).astype(BF)),
                "xk": _pack8(
                    np.ascontiguousarray(np.asarray(inputs["key"])[b].T).astype(BF)
                ),
                "xv": _pack8(
                    np.ascontiguousarray(np.asarray(inputs["value"])[b].T).astype(BF)
                ),
                "wq": wqp,
                "wk": wkp,
                "wv": wvp,
                "wo": wop,
                "bq": bqp,
                "bk": bkp,
                "bvA": bvA,
                "bo": bo,
                "kfeat": kfeat,
                "qfeat": qfeat,
            }
        )
    return in_maps


def _run(inputs, trace=False, **kw):
    from concourse.bass_utils import run_bass_kernel_spmd

    nc = _build()
    in_maps = _make_in_maps(inputs)
    res = run_bass_kernel_spmd(
        nc, in_maps, core_ids=list(range(NCORES)), trace=trace, **kw
    )
    out = np.zeros((B, T, D), np.float32)
    for c in range(NCORES):
        b, p = c // 4, c % 4
        out[b, TQ * p : TQ * (p + 1)] = res.results[c]["out"]
    return out, res


def kernel(**inputs):
    out, _ = _run(inputs)
    return out


# revision 27
# speedup vs baseline: 1.1516x; 1.0259x over previous
"""Distributed Trainium2 kernel for CustomMultiHeadAttentionStoich (v3).

Sharding (8 cores): core c = (batch b=c//4, query slice p=c%4, 512 queries).
No collectives: each core computes the FULL K^T and V for its batch locally,
which removes the baseline's two serialized AllGathers (+barrier, ~160us).

All matmul operands are bf16: fp8 was measured to cost 5-8% output error
here (the softmax concentrates on few keys, so quantization noise does not
average out), far over the 2% budget.

Engine budget per core (at the 2.4 GHz sustained PE p-state):
 - PE ~260us: Q/K/V projections, scores (96-row contraction: 64 head dims +
   32 SVD feature rows for the stoichiometric bias), AV, out-projection.
 - ACT ~146us: exclusively exp over [128,2,512] two-PSUM-bank score pairs,
   with the 1/8 softmax scale folded into the activation scale operand.
 - DVE: staging copies, bias adds, reciprocal, normalization multiplies.

SBUF is tight: kat (per-head K^T+features, 64KB/partition) and qat are
resident; xq/xk/wo share one streaming pool, and xv/wv stream per V piece.
V pieces for head-groups 1-3 are spread one per attention slot, gated so a
group's writes are only emitted after the pool's reuse-distance readers.

Attention is software-pipelined flat over (head, chunk-pair) slots with the
AV matmuls lagging one slot so the in-order PE queue never waits on the exp
of the slot it just issued.
"""

import sys

sys.path.insert(0, "/opt/trn_rl_repo")

import numpy as np
import ml_dtypes

BF = ml_dtypes.bfloat16

B, T, D, H, DH = 2, 2048, 1024, 16, 64
NCORES = 8
TQ = 512  # queries per core
R = 16  # SVD rank per clamp-kernel half
AUG = 2 * R
CP = DH + AUG  # contraction rows for the scores matmul (96)
NGRID = 1024  # SVD grid
KC = T // 128  # 16 key chunks
VG = 65  # V columns per head incl. ones column
NCP = KC // 2  # 8 chunk pairs

_state = {}


def _features():
    """Rank-R SVD features of rc(x,y)=clip(x-y,0,0.2) on [0,1]^2."""
    if "grid" not in _state:
        g = (np.arange(NGRID) + 0.5) / NGRID
        M = np.clip(g[:, None] - g[None, :], 0.0, 0.2)
        U, S, Vt = np.linalg.svd(M, full_matrices=False)
        sc = np.sqrt(S[:R] * NGRID)
        _state["grid"] = g
        _state["phi"] = (U[:, :R] * sc).astype(np.float64)  # [NGRID, R] phi_j(x)
        _state["psi"] = (Vt[:R].T * sc).astype(np.float64)  # [NGRID, R] psi_j(y)
    return _state["grid"], _state["phi"], _state["psi"]


def _ev(tab, x):
    g = _state["grid"]
    return np.stack([np.interp(x, g, tab[:, j]) for j in range(R)])


def _build():
    if "nc" in _state:
        return _state["nc"]

    import concourse.bass as bass
    import concourse.mybir as mybir
    import concourse.tile as tile
    from concourse import bacc

    dt = mybir.dt
    ts = bass.ts
    ds = bass.ds

    nc = bacc.Bacc(
        "TRN2",
        target_bir_lowering=False,
        debug=False,
        num_devices=NCORES,
    )

    # ---- kernel I/O (per-core shards; host pre-slices/casts) ----
    xqE = nc.dram_tensor("xq", [128, 8, TQ], dt.bfloat16, kind="ExternalInput").ap()
    xkE = nc.dram_tensor("xk", [128, 8, T], dt.bfloat16, kind="ExternalInput").ap()
    xvE = nc.dram_tensor("xv", [128, 8, T], dt.bfloat16, kind="ExternalInput").ap()
    wqE = nc.dram_tensor("wq", [128, 8, D], dt.bfloat16, kind="ExternalInput").ap()
    wkE = nc.dram_tensor("wk", [128, 8, D], dt.bfloat16, kind="ExternalInput").ap()
    wvE = nc.dram_tensor(
        "wv", [128, 8, H * VG], dt.bfloat16, kind="ExternalInput"
    ).ap()
    woE = nc.dram_tensor("wo", [128, 8, D], dt.bfloat16, kind="ExternalInput").ap()
    bqE = nc.dram_tensor("bq", [128, 8, 1], dt.float32, kind="ExternalInput").ap()
    bkE = nc.dram_tensor("bk", [128, 8, 1], dt.float32, kind="ExternalInput").ap()
    bvE = nc.dram_tensor("bvA", [1, H * VG], dt.bfloat16, kind="ExternalInput").ap()
    boE = nc.dram_tensor("bo", [1, D], dt.bfloat16, kind="ExternalInput").ap()
    kfE = nc.dram_tensor("kfeat", [AUG, T], dt.bfloat16, kind="ExternalInput").ap()
    qfE = nc.dram_tensor(
        "qfeat", [H * AUG, TQ], dt.bfloat16, kind="ExternalInput"
    ).ap()
    outE = nc.dram_tensor("out", [TQ, D], dt.float32, kind="ExternalOutput").ap()

    Exp = mybir.ActivationFunctionType.Exp

    with tile.TileContext(nc) as tc:
        with (
            tc.tile_pool(name="consts", bufs=1) as consts,
            tc.tile_pool(name="xs", bufs=2) as xsp,  # xq, xk/xv chunks, wo halves
            tc.tile_pool(name="wqs", bufs=4) as wqsp,  # wq streamed per dc
            tc.tile_pool(name="ehp", bufs=3) as ehp,  # exp chunk-pair tiles
            tc.tile_pool(name="stage", bufs=2) as stp,
            tc.tile_pool(name="sty", bufs=2) as styp,
            tc.tile_pool(name="nrm", bufs=2) as nrm,
            tc.tile_pool(name="nrm1", bufs=1) as nrm1,
            tc.tile_pool(name="psA", bufs=2, space="PSUM") as psA,
            tc.tile_pool(name="psS", bufs=2, space="PSUM") as psS,
            tc.tile_pool(name="psAV", bufs=2, space="PSUM") as psAV,
        ):
            # ---- resident constants ----
            wk_sb = consts.tile([128, 8, D], dt.bfloat16, tag="wk", name="wk")
            bq_sb = consts.tile([128, 8, 1], dt.float32, tag="bq", name="bq")
            bk_sb = consts.tile([128, 8, 1], dt.float32, tag="bk", name="bk")
            bv_sb = consts.tile([1, H * VG], dt.bfloat16, tag="bv", name="bv")
            bo_sb = consts.tile([1, D], dt.bfloat16, tag="bo", name="bo")
            ones_sb = consts.tile([1, 128], dt.bfloat16, tag="ones", name="ones")
            wv_sb = consts.tile([128, 8, H * VG], dt.bfloat16, tag="wv", name="wv")

            nc.gpsimd.dma_start(out=bq_sb, in_=bqE)
            nc.gpsimd.dma_start(out=bk_sb, in_=bkE)
            nc.gpsimd.dma_start(out=bv_sb, in_=bvE)
            nc.gpsimd.dma_start(out=bo_sb, in_=boE)
            nc.vector.memset(ones_sb, 1.0)

            qat = [
                consts.tile([CP, TQ], dt.bfloat16, tag=f"qat{h}", name=f"qat{h}")
                for h in range(H)
            ]
            kat = [
                consts.tile([CP, T], dt.bfloat16, tag=f"kat{h}", name=f"kat{h}")
                for h in range(H)
            ]
            aot2 = [
                consts.tile([128, TQ], dt.bfloat16, tag=f"aot{j}", name=f"aot{j}")
                for j in range(H // 2)
            ]

            vtl = consts.tile(
                [128, KC, H * VG], dt.bfloat16, tag="vtl", name="vtl"
            )
            # ---- Q projection ----
            xq_sb = xsp.tile([128, 8, TQ], dt.bfloat16, tag="xs", name="xq")
            nc.sync.dma_start(out=xq_sb, in_=xqE)
            wq_t = {}

            def load_wq(dc):
                wq_t[dc] = wqsp.tile(
                    [128, 8, 128], dt.bfloat16, tag="wq", name="wqt"
                )
                nc.gpsimd.dma_start(out=wq_t[dc], in_=wqE[:, :, ts(dc, 128)])

            for dc in range(4):
                load_wq(dc)
            nc.gpsimd.dma_start(out=wv_sb, in_=wvE)
            nc.sync.dma_start(out=wk_sb, in_=wkE)
            for dc in range(8):
                if dc + 4 < 8:
                    load_wq(dc + 4)
                ps = psA.tile([128, TQ], dt.float32, tag="mm", name="mmq")
                for kc in range(8):
                    nc.tensor.matmul(
                        ps,
                        lhsT=wq_t[dc][:, kc, :],
                        rhs=xq_sb[:, kc, :],
                        start=(kc == 0),
                        stop=(kc == 7),
                    )
                stg = stp.tile([128, TQ], dt.bfloat16, tag="stg", name="stq")
                nc.vector.tensor_scalar_add(stg, ps, bq_sb[:, dc, :])
                nc.sync.dma_start(out=qat[2 * dc][0:DH, :], in_=stg[0:DH, :])
                nc.sync.dma_start(out=qat[2 * dc + 1][0:DH, :], in_=stg[DH:128, :])

            # ---- K^T projection (full 2048 keys), xk streamed per 512 keys --
            for tc_i in range(4):
                xk_t = xsp.tile([128, 8, 512], dt.bfloat16, tag="xs", name="xkt")
                nc.scalar.dma_start(out=xk_t, in_=xkE[:, :, ts(tc_i, 512)])
                for dc in range(8):
                    ps = psA.tile([128, 512], dt.float32, tag="mm", name="mmk")
                    for kc in range(8):
                        nc.tensor.matmul(
                            ps,
                            lhsT=wk_sb[:, kc, ts(dc, 128)],
                            rhs=xk_t[:, kc, :],
                            start=(kc == 0),
                            stop=(kc == 7),
                        )
                    nc.vector.tensor_scalar_add(
                        kat[2 * dc][0:DH, ts(tc_i, 512)],
                        ps[0:DH, :],
                        bk_sb[0:DH, dc, :],
                    )
                    stg = stp.tile([128, 512], dt.bfloat16, tag="stg", name="stk")
                    nc.vector.tensor_scalar_add(
                        stg[DH:128, :], ps[DH:128, :], bk_sb[DH:128, dc, :]
                    )
                    nc.sync.dma_start(
                        out=kat[2 * dc + 1][0:DH, ts(tc_i, 512)], in_=stg[DH:128, :]
                    )

            # ---- V projection (prelude, xv streamed once) -> vdram ----
            for t4 in range(4):
                xv_t = xsp.tile([128, 8, 512], dt.bfloat16, tag="xs", name="xvt")
                nc.scalar.dma_start(out=xv_t, in_=xvE[:, :, ts(t4, 512)])
                for sub in range(4):
                    tc_i = 4 * t4 + sub
                    for g4 in range(4):
                        ps = psA.tile([128, 512], dt.float32, tag="mm", name="mmv")
                        for kc in range(8):
                            nc.tensor.matmul(
                                ps[:, 0 : 4 * VG],
                                lhsT=xv_t[:, kc, ts(sub, 128)],
                                rhs=wv_sb[:, kc, ts(g4, 4 * VG)],
                                start=(kc == 0),
                                stop=False,
                            )
                        nc.tensor.matmul(
                            ps[:, 0 : 4 * VG],
                            lhsT=ones_sb[:, :],
                            rhs=bv_sb[:, ts(g4, 4 * VG)],
                            start=False,
                            stop=True,
                        )
                        nc.vector.tensor_copy(
                            vtl[:, tc_i, ts(g4, 4 * VG)], ps[:, 0 : 4 * VG]
                        )

            for h in range(H):
                nc.gpsimd.dma_start(out=kat[h][DH:CP, :], in_=kfE)
                nc.gpsimd.dma_start(out=qat[h][DH:CP, :], in_=qfE[ds(AUG * h, AUG), :])

            # ---- attention: flat pipeline over (head, chunk-pair) ----
            av_t = [None] * H
            eh_slot = {}
            pend_av = None  # (h, cp)
            pend_norm = None  # h, pre-norm due at next head's first slot
            pend_norm_pe = None  # h, PE broadcast due a few slots later
            norm_rcb = {}

            def emit_av(h, cp_i):
                eh2 = eh_slot.pop((h, cp_i))
                for j in range(2):
                    nc.tensor.matmul(
                        av_t[h],
                        lhsT=vtl[:, 2 * cp_i + j, ds(VG * h, VG)],
                        rhs=eh2[:, j, :],
                        start=(cp_i == 0 and j == 0),
                        stop=(cp_i == NCP - 1 and j == 1),
                    )

            def emit_norm_pre(h):
                # den: PSUM row 64 -> SBUF (lane-aligned) -> DMA to partition 0,
                # then reciprocal at partition 0 (recip can't read PSUM@64 on HW)
                den65 = nrm1.tile([VG, TQ], dt.float32, tag="den65", name="den65")
                nc.vector.tensor_copy(den65[DH:VG, :], av_t[h][DH:VG, :])
                den0 = nrm1.tile([1, TQ], dt.float32, tag="den0", name="den0")
                nc.sync.dma_start(out=den0, in_=den65[DH:VG, :])
                rcf = nrm1.tile([1, TQ], dt.float32, tag="rcf", name="rcf")
                nc.vector.reciprocal_approx_fast(rcf, den0)
                rcb0 = nrm.tile([1, TQ], dt.bfloat16, tag="rcb0", name="rcb0")
                nc.vector.tensor_copy(rcb0, rcf)
                norm_rcb[h] = rcb0

            def emit_norm_pe(h):
                # PE broadcast of rcb over 64 partitions, then DVE multiply
                ps_av = av_t[h]
                psb = psA.tile([DH, TQ], dt.float32, tag="mm", name="mmb")
                nc.tensor.matmul(
                    psb,
                    lhsT=ones_sb[:, 0:DH],
                    rhs=norm_rcb[h],
                    start=True,
                    stop=True,
                )
                bcb = nrm.tile([DH, TQ], dt.bfloat16, tag="bcb", name="bcb")
                nc.vector.tensor_copy(bcb, psb)
                if h % 2 == 0:
                    nc.vector.tensor_mul(
                        aot2[h // 2][0:DH, :], ps_av[0:DH, :], bcb
                    )
                else:
                    ot = nrm.tile([DH, TQ], dt.bfloat16, tag="ot", name="ot")
                    nc.vector.tensor_mul(ot, ps_av[0:DH, :], bcb)
                    nc.sync.dma_start(out=aot2[h // 2][DH:128, :], in_=ot)

            for h in range(H):
                av_t[h] = psAV.tile([VG, TQ], dt.float32, tag="av", name="av")
                for cp_i in range(NCP):
                    ps_s = psS.tile([128, 2, TQ], dt.float32, tag="s", name="s")
                    for j in range(2):
                        nc.tensor.matmul(
                            ps_s[:, j, :],
                            lhsT=kat[h][:, ts(2 * cp_i + j, 128)],
                            rhs=qat[h],
                            start=True,
                            stop=True,
                        )
                    eh2 = ehp.tile([128, 2, TQ], dt.bfloat16, tag="eh", name="eh")
                    nc.scalar.activation(eh2, ps_s, Exp, scale=0.125)
                    eh_slot[(h, cp_i)] = eh2
                    if pend_av is not None:
                        emit_av(*pend_av)
                        pend_av = None
                        if pend_norm is not None:
                            emit_norm_pre(pend_norm)
                            pend_norm_pe, pend_norm = pend_norm, None
                    if pend_norm_pe is not None and cp_i >= 3:
                        emit_norm_pe(pend_norm_pe)
                        pend_norm_pe = None
                    pend_av = (h, cp_i)
                pend_norm = h
            emit_av(*pend_av)
            emit_norm_pre(pend_norm)
            emit_norm_pe(pend_norm)

            # ---- output projection (2 heads per 128-row chunk) ----
            wo_t = [
                xsp.tile([128, 8, 512], dt.bfloat16, tag="xs", name=f"wo{mc}")
                for mc in range(2)
            ]
            for mc in range(2):
                nc.sync.dma_start(out=wo_t[mc], in_=woE[:, :, ts(mc, 512)])
            for qc in range(4):
                for mc in range(2):
                    ps_y = psA.tile([128, 512], dt.float32, tag="mm", name="mmy")
                    for j in range(H // 2):
                        nc.tensor.matmul(
                            ps_y,
                            lhsT=aot2[j][:, ts(qc, 128)],
                            rhs=wo_t[mc][:, j, :],
                            start=(j == 0),
                            stop=False,
                        )
                    nc.tensor.matmul(
                        ps_y,
                        lhsT=ones_sb[:, :],
                        rhs=bo_sb[:, ts(mc, 512)],
                        start=False,
                        stop=True,
                    )
                    yst = styp.tile([128, 512], dt.float32, tag="sty", name="sty")
                    nc.vector.tensor_copy(yst, ps_y)
                    nc.sync.dma_start(out=outE[ts(qc, 128), ts(mc, 512)], in_=yst)

    nc.compile()
    _state["nc"] = nc
    return nc


def _pack8(mat):
    """[1024, N] -> [128, 8, N] with element [p, g, n] = mat[g*128+p, n]."""
    n = mat.shape[1]
    return np.ascontiguousarray(mat.reshape(8, 128, n).transpose(1, 0, 2))


def _make_in_maps(inputs):
    _features()
    gamma = float(np.float32(inputs["gamma"]))
    delta = float(np.float32(inputs["delta"]))
    ap_ = np.asarray(inputs["alpha_pos"], np.float64)
    an_ = np.asarray(inputs["alpha_neg"], np.float64)

    # exp() applies scale=1/8; gamma folds into Wq; qfeat compensates by 8.
    wqp = _pack8((np.asarray(inputs["Wq"], np.float64).T * gamma).astype(BF))
    bq = (np.asarray(inputs["bq"], np.float64) * gamma).astype(np.float32)
    bqp = np.ascontiguousarray(bq.reshape(8, 128).T)[:, :, None]
    wkp = _pack8(np.ascontiguousarray(np.asarray(inputs["Wk"]).T).astype(BF))
    bkp = np.ascontiguousarray(
        np.asarray(inputs["bk"], np.float32).reshape(8, 128).T
    )[:, :, None]
    wop = _pack8(np.ascontiguousarray(np.asarray(inputs["Wo"]).T).astype(BF))
    bo = np.asarray(inputs["bo"], np.float32)[None, :].astype(BF)

    wvT = np.asarray(inputs["Wv"], np.float64).T
    wvP = np.zeros((D, H * VG), np.float64)
    bvA = np.zeros((1, H * VG), np.float64)
    for h in range(H):
        wvP[:, VG * h : VG * h + DH] = wvT[:, DH * h : DH * h + DH]
        bvA[0, VG * h : VG * h + DH] = np.asarray(inputs["bv"], np.float64)[
            DH * h : DH * h + DH
        ]
        bvA[0, VG * h + DH] = 1.0
    wvp = _pack8(wvP.astype(BF))
    bvA = bvA.astype(BF)

    phi, psi = _state["phi"], _state["psi"]
    frac = np.asarray(inputs["frac"], np.float64)

    in_maps = []
    for c in range(NCORES):
        b, p = c // 4, c % 4
        fb = frac[b]
        fq = fb[TQ * p : TQ * (p + 1)]
        kfeat = np.concatenate([_ev(phi, fb), _ev(psi, fb)], 0).astype(BF)
        qfeat = np.zeros((H * AUG, TQ), np.float64)
        for h in range(H):
            a_h = 8.0 * delta * ap_[h] / NGRID
            b_h = -8.0 * delta * an_[h] / NGRID
            qfeat[AUG * h : AUG * h + R] = a_h * _ev(psi, fq)
            qfeat[AUG * h + R : AUG * (h + 1)] = b_h * _ev(phi, fq)
        qfeat = qfeat.astype(BF)

        xq = np.asarray(inputs["query"])[b, TQ * p : TQ * (p + 1)]
        in_maps.append(
            {
                "xq": _pack8(np.ascontiguousarray(xq.T).astype(BF)),
                "xk": _pack8(
                    np.ascontiguousarray(np.asarray(inputs["key"])[b].T).astype(BF)
                ),
                "xv": _pack8(
                    np.ascontiguousarray(np.asarray(inputs["value"])[b].T).astype(BF)
                ),
                "wq": wqp,
                "wk": wkp,
                "wv": wvp,
                "wo": wop,
                "bq": bqp,
                "bk": bkp,
                "bvA": bvA,
                "bo": bo,
                "kfeat": kfeat,
                "qfeat": qfeat,
            }
        )
    return in_maps


def _run(inputs, trace=False, **kw):
    from concourse.bass_utils import run_bass_kernel_spmd

    nc = _build()
    in_maps = _make_in_maps(inputs)
    res = run_bass_kernel_spmd(
        nc, in_maps, core_ids=list(range(NCORES)), trace=trace, **kw
    )
    out = np.zeros((B, T, D), np.float32)
    for c in range(NCORES):
        b, p = c // 4, c % 4
        out[b, TQ * p : TQ * (p + 1)] = res.results[c]["out"]
    return out, res


def kernel(**inputs):
    out, _ = _run(inputs)
    return out
